# revision 1
# baseline (speedup 1.0000x reference)
"""GAT layer (PyG GATConv defaults) on 8 Trainium2 NeuronCores.

Strategy:
- Nodes sharded across 8 cores (6250 each, padded to 6272 = 49x128 windows);
  edges partitioned by destination core, sorted by destination, grouped into
  128-node destination windows, padded to 128-edge chunks.
- Program A (per-core shard): a_src/a_dst node projections (x @ (W@att)).
  Host expands them to per-edge alpha = a_src[src] + a_dst[dst] arrays.
- Program B: phase 1 rebuilds h = x@W as an f16 row table G[50176, 256]
  (replicated on every core); phase 2 per destination window dma_gathers
  h[src] rows (split lo/hi tables for int16 indices), builds per-head
  weighted one-hot matrices ohw[e,n] = exp(lrelu(alpha)) * (dst==n) on the
  vector engine (dual-op tensor_scalar), and accumulates both the weighted
  message sum and the softmax denominators with PE matmuls into PSUM.
  Epilogue normalizes, adds bias, applies relu.
"""
import os
import sys

sys.path.insert(0, "/opt/trn_rl_repo")
os.environ.setdefault("MYCRO_LOCAL_CACHE", "1")

import numpy as np

N = 50000
E = 800000
IN_CH = 256
OUT_CH = 64
HEADS = 4
NEG = 0.2
M = 8
NPC = 6250
NPCP = 6272
NW = 49
NG = NPCP * M  # 50176
HALF = NG // 2  # 25088
P = 128
NT = NG // P  # 392 node tiles

_cache = {}


def _build_prog_a():
    import concourse.mybir as mybir
    import concourse.tile as tile
    from concourse import bacc

    f16 = mybir.dt.float16
    f32 = mybir.dt.float32
    nc = bacc.Bacc("TRN2", target_bir_lowering=False, debug=False, num_devices=M)
    xs = nc.dram_tensor("xs", [IN_CH, NPCP], f16, kind="ExternalInput")
    wa = nc.dram_tensor("wa", [IN_CH, 2 * HEADS], f16, kind="ExternalInput")
    ao = nc.dram_tensor("ao", [NPCP, 2 * HEADS], f32, kind="ExternalOutput")
    with tile.TileContext(nc) as tc:
        with (
            tc.tile_pool(name="sbuf", bufs=3) as sb,
            tc.tile_pool(name="psum", bufs=2, space="PSUM") as ps,
        ):
            wa_sb = sb.tile([P, 2, 2 * HEADS], f16, tag="wa")
            for k in range(2):
                nc.sync.dma_start(
                    out=wa_sb[:, k, :], in_=wa.ap()[P * k : P * (k + 1), :]
                )
            GRP = 7  # 49 tiles = 7 groups of 7
            for t0 in range(0, NW, GRP):
                xt = sb.tile([P, 2, GRP * P], f16, tag="xt")
                for k in range(2):
                    nc.sync.dma_start(
                        out=xt[:, k, :],
                        in_=xs.ap()[P * k : P * (k + 1), P * t0 : P * (t0 + GRP)],
                    )
                o = sb.tile([P, GRP, 2 * HEADS], f32, tag="aout")
                for tt in range(GRP):
                    pt = ps.tile([P, 2 * HEADS], f32, tag="pa")
                    for k in range(2):
                        nc.tensor.matmul(
                            out=pt[:],
                            lhsT=xt[:, k, P * tt : P * (tt + 1)],
                            rhs=wa_sb[:, k, :],
                            start=(k == 0),
                            stop=(k == 1),
                        )
                    nc.vector.tensor_copy(out=o[:, tt, :], in_=pt[:])
                nc.sync.dma_start(
                    out=ao.ap()[P * t0 : P * (t0 + GRP), :].rearrange(
                        "(t p) c -> p t c", p=P
                    ),
                    in_=o[:, :, :],
                )
    nc.compile()
    return nc


def _build_prog_b(C0s, C1s):
    import concourse.mybir as mybir
    import concourse.tile as tile
    from concourse import bacc

    f16 = mybir.dt.float16
    f32 = mybir.dt.float32
    i16 = mybir.dt.int16
    L0, L1 = int(sum(C0s)), int(sum(C1s))
    CT = L0 + L1
    Cmax = int(max(C0s[w] + C1s[w] for w in range(NW)))

    nc = bacc.Bacc("TRN2", target_bir_lowering=False, debug=False, num_devices=M)
    xT = nc.dram_tensor("xT", [IN_CH, NG], f16, kind="ExternalInput")
    wt = nc.dram_tensor("wt", [IN_CH, IN_CH], f16, kind="ExternalInput")
    idx0 = nc.dram_tensor("idx0", [P, L0 * 8], i16, kind="ExternalInput")
    idx1 = nc.dram_tensor("idx1", [P, L1 * 8], i16, kind="ExternalInput")
    dstr = nc.dram_tensor("dstr", [P, CT], f32, kind="ExternalInput")
    alp = nc.dram_tensor("alp", [P, CT * 4], f32, kind="ExternalInput")
    iota = nc.dram_tensor("iota", [P, P], f16, kind="ExternalInput")
    brep = nc.dram_tensor("brep", [P, IN_CH], f32, kind="ExternalInput")
    out = nc.dram_tensor("out", [NPCP, IN_CH], f32, kind="ExternalOutput")
    G = nc.dram_tensor("G", [NG, IN_CH], f16, kind="Internal")

    with tile.TileContext(nc) as tc:
        with (
            tc.tile_pool(name="sbuf", bufs=3) as sb,
            tc.tile_pool(name="cst", bufs=1) as cst,
            tc.tile_pool(name="psum", bufs=2, space="PSUM") as ps,
        ):
            w_sb = cst.tile([P, 2, IN_CH], f16, tag="w")
            for k in range(2):
                nc.sync.dma_start(
                    out=w_sb[:, k, :], in_=wt.ap()[P * k : P * (k + 1), :]
                )
            iota_sb = cst.tile([P, P], f16, tag="iota")
            nc.sync.dma_start(out=iota_sb[:], in_=iota.ap()[:, :])
            brep_sb = cst.tile([P, IN_CH], f32, tag="brep")
            nc.sync.dma_start(out=brep_sb[:], in_=brep.ap()[:, :])
            ones_sb = cst.tile([P, 1], f16, tag="ones")
            nc.vector.memset(ones_sb[:], 1.0)

            # ---- phase 1: G[n] = (x @ W)[n] in f16, 8 node-tiles per group ----
            GRP = 8
            for t0 in range(0, NT, GRP):
                xt = sb.tile([P, 2, GRP * P], f16, tag="xt")
                for k in range(2):
                    nc.sync.dma_start(
                        out=xt[:, k, :],
                        in_=xT.ap()[P * k : P * (k + 1), P * t0 : P * (t0 + GRP)],
                    )
                hs = sb.tile([P, GRP, IN_CH], f16, tag="h")
                for tt in range(GRP):
                    pt = ps.tile([P, IN_CH], f32, tag="p1")
                    for k in range(2):
                        nc.tensor.matmul(
                            out=pt[:],
                            lhsT=xt[:, k, P * tt : P * (tt + 1)],
                            rhs=w_sb[:, k, :],
                            start=(k == 0),
                            stop=(k == 1),
                        )
                    nc.scalar.activation(
                        out=hs[:, tt, :], in_=pt[:],
                        func=mybir.ActivationFunctionType.Copy,
                    )
                nc.sync.dma_start(
                    out=G.ap()[P * t0 : P * (t0 + GRP), :].rearrange(
                        "(t p) c -> p t c", p=P
                    ),
                    in_=hs[:, :, :],
                )

            tc.strict_bb_all_engine_barrier()

            # ---- phase 2: per destination window ----
            idx0_sb = cst.tile([P, L0 * 8], i16, tag="idx0")
            nc.sync.dma_start(out=idx0_sb[:], in_=idx0.ap()[:, :])
            idx1_sb = cst.tile([P, L1 * 8], i16, tag="idx1")
            nc.sync.dma_start(out=idx1_sb[:], in_=idx1.ap()[:, :])
            dstr_sb = cst.tile([P, CT], f32, tag="dstr")
            nc.sync.dma_start(out=dstr_sb[:], in_=dstr.ap()[:, :])
            alp_sb = cst.tile([P, CT * 4], f32, tag="alp")
            nc.sync.dma_start(out=alp_sb[:], in_=alp.ap()[:, :])

            o0 = o1 = cc = 0
            for w in range(NW):
                c0, c1 = int(C0s[w]), int(C1s[w])
                C = c0 + c1
                gt = sb.tile([P, Cmax, IN_CH], f16, tag="gt")
                nc.gpsimd.dma_gather(
                    out_ap=gt[:, 0:c0, :],
                    in_ap=G.ap()[0:HALF, :],
                    idxs_ap=idx0_sb[:, o0 * 8 : (o0 + c0) * 8],
                    num_idxs=c0 * P,
                    num_idxs_reg=c0 * P,
                    elem_size=IN_CH,
                    single_packet=(c0 * P <= 1024),
                )
                nc.gpsimd.dma_gather(
                    out_ap=gt[:, c0:C, :],
                    in_ap=G.ap()[HALF:NG, :],
                    idxs_ap=idx1_sb[:, o1 * 8 : (o1 + c1) * 8],
                    num_idxs=c1 * P,
                    num_idxs_reg=c1 * P,
                    elem_size=IN_CH,
                    single_packet=(c1 * P <= 1024),
                )
                al_ap = alp_sb[:, cc * 4 : (cc + C) * 4]
                el = sb.tile([P, Cmax * 4], f32, tag="el")
                nc.vector.tensor_scalar(
                    out=el[:, : C * 4], in0=al_ap, scalar1=NEG, scalar2=None,
                    op0=mybir.AluOpType.mult,
                )
                nc.vector.tensor_tensor(
                    out=el[:, : C * 4], in0=al_ap, in1=el[:, : C * 4],
                    op=mybir.AluOpType.max,
                )
                ev = sb.tile([P, Cmax * 4], f32, tag="ev")
                nc.scalar.activation(
                    out=ev[:, : C * 4], in_=el[:, : C * 4],
                    func=mybir.ActivationFunctionType.Exp,
                )

                ptm = ps.tile([P, IN_CH], f32, tag="pm")
                ptd = ps.tile([P, HEADS], f32, tag="pd")
                for h in range(HEADS):
                    for c in range(C):
                        oh = sb.tile([P, P], f16, tag="oh")
                        nc.vector.tensor_scalar(
                            out=oh[:],
                            in0=iota_sb[:],
                            scalar1=dstr_sb[:, cc + c : cc + c + 1],
                            scalar2=ev[:, 4 * c + h : 4 * c + h + 1],
                            op0=mybir.AluOpType.is_equal,
                            op1=mybir.AluOpType.mult,
                        )
                        nc.tensor.matmul(
                            out=ptm[:, OUT_CH * h : OUT_CH * (h + 1)],
                            lhsT=oh[:],
                            rhs=gt[:, c, OUT_CH * h : OUT_CH * (h + 1)],
                            start=(c == 0),
                            stop=(c == C - 1),
                        )
                        nc.tensor.matmul(
                            out=ptd[:, h : h + 1],
                            lhsT=oh[:],
                            rhs=ones_sb[:, 0:1],
                            start=(c == 0),
                            stop=(c == C - 1),
                        )

                den = sb.tile([P, HEADS], f32, tag="den")
                nc.vector.tensor_scalar(
                    out=den[:], in0=ptd[:], scalar1=1e-16, scalar2=None,
                    op0=mybir.AluOpType.add,
                )
                rcp = sb.tile([P, HEADS], f32, tag="rcp")
                nc.vector.reciprocal(out=rcp[:], in_=den[:])
                osb = sb.tile([P, IN_CH], f32, tag="osb")
                nc.vector.tensor_tensor(
                    out=osb[:].rearrange("p (h c) -> p h c", h=HEADS),
                    in0=ptm[:].rearrange("p (h c) -> p h c", h=HEADS),
                    in1=rcp[:].unsqueeze(2).to_broadcast([P, HEADS, OUT_CH]),
                    op=mybir.AluOpType.mult,
                )
                nc.vector.tensor_tensor(
                    out=osb[:], in0=osb[:], in1=brep_sb[:],
                    op=mybir.AluOpType.add,
                )
                nc.vector.tensor_scalar(
                    out=osb[:], in0=osb[:], scalar1=0.0, scalar2=None,
                    op0=mybir.AluOpType.max,
                )
                nc.sync.dma_start(
                    out=out.ap()[P * w : P * (w + 1), :], in_=osb[:]
                )
                o0 += c0
                o1 += c1
                cc += C
    nc.compile()
    return nc


def _wrap16_concat(flat_vals, counts):
    """Per-window wrap-16 idx layout, concatenated along the free dim,
    replicated to 128 partitions. flat_vals: int16 slot values, window w
    occupying flat[128*cum[w] : 128*cum[w]+128*counts[w]]."""
    total_cols = int(sum(counts)) * 8
    arr = np.zeros((16, total_cols), np.int16)
    off = 0
    col = 0
    for c in counts:
        c = int(c)
        blk = flat_vals[off : off + c * P].reshape(c * 8, 16).T
        arr[:, col : col + c * 8] = blk
        off += c * P
        col += c * 8
    return np.tile(arr, (8, 1))


def _prep_edges(edge_index, a_all):
    """Build per-core phase-2 input arrays from edge_index and the per-node
    attention projections a_all [NG, 8] (cols 0:4 a_src, 4:8 a_dst)."""
    src = np.concatenate([edge_index[0], np.arange(N, dtype=np.int64)]).astype(
        np.int64
    )
    dst = np.concatenate([edge_index[1], np.arange(N, dtype=np.int64)]).astype(
        np.int64
    )
    srcp = src + (NPCP - NPC) * (src // NPC)
    core = dst // NPC
    dloc = dst % NPC
    dwin = dloc // P
    drel = dloc % P
    dstp = NPCP * core + dloc
    half = (srcp >= HALF).astype(np.int64)  # 0 = lo, 1 = hi

    # per (core, window, half) counts -> shared static schedule
    cnt = np.zeros((M, NW, 2), np.int64)
    np.add.at(cnt, (core, dwin, half), 1)
    chunks = -(-cnt // P)  # ceil
    C0s = chunks[:, :, 0].max(axis=0)
    C1s = chunks[:, :, 1].max(axis=0)
    Ccomb = C0s + C1s
    base0 = np.concatenate([[0], np.cumsum(C0s)]) * P
    base1 = np.concatenate([[0], np.cumsum(C1s)]) * P
    basec = np.concatenate([[0], np.cumsum(Ccomb)]) * P
    S0, S1, SC = int(base0[-1]), int(base1[-1]), int(basec[-1])

    per_core = []
    for c in range(M):
        m = core == c
        ew, eh, er = dwin[m], half[m], drel[m]
        es, ed = srcp[m], dstp[m]
        order = np.lexsort((eh, ew))
        ew, eh, er, es, ed = (a[order] for a in (ew, eh, er, es, ed))
        gid = ew * 2 + eh
        # rank within (window, half) group
        uniq, start, cnts = np.unique(gid, return_index=True, return_counts=True)
        gstart = np.zeros(2 * NW, np.int64)
        gstart[uniq] = start
        rank = np.arange(len(gid)) - gstart[gid]
        # half-local slot (for idx arrays)
        shalf = np.where(eh == 0, base0[ew], base1[ew]) + rank
        # combined slot (for dstrel / alpha)
        scomb = basec[ew] + np.where(eh == 0, 0, C0s[ew] * P) + rank

        idx0f = np.zeros(S0, np.int16)
        idx1f = np.zeros(S1, np.int16)
        lo_m = eh == 0
        idx0f[shalf[lo_m]] = es[lo_m].astype(np.int16)
        idx1f[shalf[~lo_m]] = (es[~lo_m] - HALF).astype(np.int16)

        drf = np.full(SC, -1.0, np.float32)
        drf[scomb] = er.astype(np.float32)
        alf = np.zeros((SC, 4), np.float32)
        alf[scomb] = a_all[es, 0:4] + a_all[ed, 4:8]

        per_core.append(
            {
                "idx0": _wrap16_concat(idx0f, C0s),
                "idx1": _wrap16_concat(idx1f, C1s),
                "dstr": np.ascontiguousarray(drf.reshape(SC // P, P).T),
                "alp": np.ascontiguousarray(
                    alf.reshape(SC // P, P, 4).transpose(1, 0, 2).reshape(P, -1)
                ),
            }
        )
    return tuple(int(v) for v in C0s), tuple(int(v) for v in C1s), per_core


def kernel(x, edge_index, W, att_src, att_dst, bias):
    from concourse.bass_utils import run_bass_kernel_spmd

    x = np.asarray(x, dtype=np.float32)
    edge_index = np.asarray(edge_index).astype(np.int64)
    W = np.asarray(W, dtype=np.float32)
    att_src = np.asarray(att_src, dtype=np.float32)
    att_dst = np.asarray(att_dst, dtype=np.float32)
    bias = np.asarray(bias, dtype=np.float32)

    # host data layout prep
    x_pad = np.zeros((NG, IN_CH), np.float32)
    for c in range(M):
        x_pad[NPCP * c : NPCP * c + NPC] = x[NPC * c : NPC * c + NPC]
    xT_f16 = np.ascontiguousarray(x_pad.T).astype(np.float16)

    A8 = np.zeros((IN_CH, 2 * HEADS), np.float32)
    for h in range(HEADS):
        A8[OUT_CH * h : OUT_CH * (h + 1), h] = att_src[h]
        A8[OUT_CH * h : OUT_CH * (h + 1), HEADS + h] = att_dst[h]
    WA8 = (W @ A8).astype(np.float16)
    W_f16 = W.astype(np.float16)
    iota = np.tile(np.arange(P, dtype=np.float16), (P, 1))
    brep = np.tile(bias.astype(np.float32), (P, 1))

    # ---- program A: per-node attention projections ----
    if "A" in _cache:
        ncA = _cache["A"]
    else:
        ncA = _cache["A"] = _build_prog_a()
    in_maps_a = [
        {
            "xs": np.ascontiguousarray(xT_f16[:, NPCP * c : NPCP * (c + 1)]),
            "wa": WA8,
        }
        for c in range(M)
    ]
    res_a = run_bass_kernel_spmd(ncA, in_maps_a, core_ids=list(range(M)))
    a_all = np.concatenate([res_a.results[c]["ao"] for c in range(M)], axis=0)

    # ---- host: expand per-edge alpha, build schedule ----
    C0s, C1s, per_core = _prep_edges(edge_index, a_all)

    key = ("B", C0s, C1s)
    if key in _cache:
        ncB = _cache[key]
    else:
        ncB = _cache[key] = _build_prog_b(C0s, C1s)
    in_maps_b = []
    for c in range(M):
        d = dict(per_core[c])
        d.update(
            {"xT": xT_f16, "wt": W_f16, "iota": iota, "brep": brep}
        )
        in_maps_b.append(d)
    res_b = run_bass_kernel_spmd(ncB, in_maps_b, core_ids=list(range(M)))

    out = np.empty((N, IN_CH), np.float32)
    for c in range(M):
        out[NPC * c : NPC * (c + 1)] = res_b.results[c]["out"][:NPC]
    _cache["last_run"] = (in_maps_a, in_maps_b)
    return out


def timed_run(np_inputs):
    """Re-run both programs with tracing; return summed max-core exec ns."""
    from concourse.bass_utils import run_bass_kernel_spmd

    if "last_run" not in _cache:
        kernel(**np_inputs)
    in_maps_a, in_maps_b = _cache["last_run"]
    ncA = _cache["A"]
    ncB = next(v for k, v in _cache.items() if isinstance(k, tuple) and k[0] == "B")
    total = 0
    for nc, im in ((ncA, in_maps_a), (ncB, in_maps_b)):
        r = run_bass_kernel_spmd(
            nc, im, core_ids=list(range(M)), trace=True
        )
        print("  exec_time_ns:", r.exec_time_ns, "trace:",
              r.instructions_and_trace[1] if r.instructions_and_trace else None)
        if r.exec_time_ns:
            total += r.exec_time_ns
    return total



# revision 7
# speedup vs baseline: 1.9156x; 1.9156x over previous
"""GAT layer (PyG GATConv defaults) on 8 Trainium2 NeuronCores.

Strategy (v2):
- Nodes sharded across 8 cores (6250 each, padded to 6272 = 49x128 windows);
  edges partitioned by destination core, grouped into 128-node destination
  windows, chunked into 128-edge chunks (lo/hi source-table halves so gather
  indices fit int16).
- Program A (per-core shard): one 264-wide matmul per node tile computes both
  h = x@W (f16) and the a_src/a_dst projections (x @ (W@att)). Host assembles
  the full padded h table G [50176, 256] from the 8 shards.
- Host: expands per-edge alpha = a_src[src] + a_dst[dst], builds the window/
  chunk schedule. Self-loops form chunk 0 of every window (identity one-hot;
  rows come from the core's own shard via contiguous DMA, not gather).
- Program B (the hot kernel): per destination window w with C chunks:
  2 dma_gathers of h[src] rows (lo/hi, spread over 4 SWDGE queues, trailing
  -1 indices skip padded slots), one batched is_equal builds all C one-hot
  matrices, one batched broadcast-multiply scales gathered rows by
  ev = exp(lrelu(alpha) - 4) (softmax shift-invariant), ev is also copied
  into 4 extra rhs columns so each chunk needs ONE 260-wide PE matmul that
  accumulates both the weighted message sum (256 cols) and the softmax
  denominators (4 cols) into PSUM. Epilogue: reciprocal, scale, +bias, relu
  (relu/cast on the scalar engine), f16 output.
"""
import os
import sys

sys.path.insert(0, "/opt/trn_rl_repo")
os.environ.setdefault("MYCRO_LOCAL_CACHE", "1")

import numpy as np

N = 50000
E = 800000
IN_CH = 256
OUT_CH = 64
HEADS = 4
NEG = 0.2
M = 8
NPC = 6250
NPCP = 6272
NW = 49
NG = NPCP * M  # 50176
HALF = NG // 2  # 25088
P = 128
EXP_SHIFT = -4.0  # ev scaled by e^-4 (softmax-invariant); keeps f16 in range

_cache = {}


def _build_prog_a():
    import concourse.mybir as mybir
    import concourse.tile as tile
    from concourse import bacc

    f16 = mybir.dt.float16
    f32 = mybir.dt.float32
    WC = 2 * HEADS + IN_CH  # 264: [a-projections | W]
    nc = bacc.Bacc("TRN2", target_bir_lowering=False, debug=False, num_devices=M)
    xs = nc.dram_tensor("xs", [IN_CH, NPCP], f16, kind="ExternalInput")
    wc = nc.dram_tensor("wc", [IN_CH, WC], f16, kind="ExternalInput")
    ao = nc.dram_tensor("ao", [NPCP, 2 * HEADS], f32, kind="ExternalOutput")
    hs = nc.dram_tensor("hs", [NPCP, IN_CH], f16, kind="ExternalOutput")
    with tile.TileContext(nc) as tc:
        with (
            tc.tile_pool(name="sbuf", bufs=3) as sb,
            tc.tile_pool(name="cst", bufs=1) as cst,
            tc.tile_pool(name="psum", bufs=4, space="PSUM") as ps,
        ):
            wc_sb = cst.tile([P, 2, WC], f16, tag="wc")
            for k in range(2):
                nc.sync.dma_start(
                    out=wc_sb[:, k, :], in_=wc.ap()[P * k : P * (k + 1), :]
                )
            GRP = 7  # 49 tiles = 7 groups of 7
            for t0 in range(0, NW, GRP):
                xt = sb.tile([P, 2, GRP * P], f16, tag="xt")
                for k in range(2):
                    nc.sync.dma_start(
                        out=xt[:, k, :],
                        in_=xs.ap()[P * k : P * (k + 1), P * t0 : P * (t0 + GRP)],
                    )
                o = sb.tile([P, GRP, 2 * HEADS], f32, tag="aout")
                hh = sb.tile([P, GRP, IN_CH], f16, tag="hout")
                for tt in range(GRP):
                    pt = ps.tile([P, WC], f32, tag="pa")
                    for k in range(2):
                        nc.tensor.matmul(
                            out=pt[:],
                            lhsT=xt[:, k, P * tt : P * (tt + 1)],
                            rhs=wc_sb[:, k, :],
                            start=(k == 0),
                            stop=(k == 1),
                        )
                    nc.vector.tensor_copy(out=o[:, tt, :], in_=pt[:, 0 : 2 * HEADS])
                    nc.scalar.activation(
                        out=hh[:, tt, :], in_=pt[:, 2 * HEADS : WC],
                        func=mybir.ActivationFunctionType.Copy,
                    )
                nc.sync.dma_start(
                    out=ao.ap()[P * t0 : P * (t0 + GRP), :].rearrange(
                        "(t p) c -> p t c", p=P
                    ),
                    in_=o[:, :, :],
                )
                nc.sync.dma_start(
                    out=hs.ap()[P * t0 : P * (t0 + GRP), :].rearrange(
                        "(t p) c -> p t c", p=P
                    ),
                    in_=hh[:, :, :],
                )
    nc.compile()
    return nc


SCALE4D = True  # 4D tensor_tensor broadcast; flip False for per-head fallback


def _build_prog_b(C0s, C1s):
    import concourse.mybir as mybir
    import concourse.tile as tile
    from concourse import bacc

    f16 = mybir.dt.float16
    f32 = mybir.dt.float32
    i16 = mybir.dt.int16
    L0, L1 = int(sum(C0s)), int(sum(C1s))
    CWs = [1 + int(C0s[w]) + int(C1s[w]) for w in range(NW)]
    CT = int(sum(CWs))
    Cmax = max(CWs)
    C0m = max(1, int(max(C0s)))
    C1m = max(1, int(max(C1s)))
    RH = IN_CH + HEADS  # 260 rhs cols: 256 msg + 4 denominator

    nc = bacc.Bacc("TRN2", target_bir_lowering=False, debug=False, num_devices=M)
    G = nc.dram_tensor("G", [NG, IN_CH], f16, kind="ExternalInput")
    gselfd = nc.dram_tensor("gself", [NPCP, IN_CH], f16, kind="ExternalInput")
    idx0 = nc.dram_tensor("idx0", [P, max(1, L0 * 8)], i16, kind="ExternalInput")
    idx1 = nc.dram_tensor("idx1", [P, max(1, L1 * 8)], i16, kind="ExternalInput")
    dstr = nc.dram_tensor("dstr", [P, CT], f16, kind="ExternalInput")
    alp = nc.dram_tensor("alp", [P, CT * 4], f32, kind="ExternalInput")
    iota = nc.dram_tensor("iota", [P, Cmax * P], f16, kind="ExternalInput")
    brep = nc.dram_tensor("brep", [P, IN_CH], f32, kind="ExternalInput")
    outd = nc.dram_tensor("out", [NPCP, IN_CH], f16, kind="ExternalOutput")

    def scale(out4, in4, evs):
        # out4/in4: 4D [P, c, 4, 64] views; evs: [P, c*4] ev slice
        c = out4.shape[1]
        ev4 = (
            evs.rearrange("p (c h) -> p c h", h=HEADS)
            .unsqueeze(3)
            .to_broadcast([P, c, HEADS, OUT_CH])
        )
        nc.vector.tensor_tensor(out=out4, in0=in4, in1=ev4, op=mybir.AluOpType.mult)

    def scale_fallback(gts_slice, in3, evs, c):
        # per-head 3D: gts_slice/in3 [P, c, 256]; evs [P, c*4]
        ev3 = evs.rearrange("p (c h) -> p c h", h=HEADS)
        for h in range(HEADS):
            nc.vector.tensor_tensor(
                out=gts_slice[:, :, OUT_CH * h : OUT_CH * (h + 1)],
                in0=in3[:, :, OUT_CH * h : OUT_CH * (h + 1)],
                in1=ev3[:, :, h : h + 1].to_broadcast([P, c, OUT_CH]),
                op=mybir.AluOpType.mult,
            )

    def do_scale(gts, o, c, src3, evs):
        # gts [P, Cmax, RH]; write chunks [o, o+c) cols 0:256 = src3 * ev
        if SCALE4D:
            scale(
                gts[:, o : o + c, 0:IN_CH].rearrange(
                    "p c (h j) -> p c h j", h=HEADS
                ),
                src3.rearrange("p c (h j) -> p c h j", h=HEADS),
                evs,
            )
        else:
            scale_fallback(gts[:, o : o + c, 0:IN_CH], src3, evs, c)

    with tile.TileContext(nc) as tc:
        with (
            tc.tile_pool(name="sbuf", bufs=3) as sb,
            tc.tile_pool(name="cst", bufs=1) as cst,
            tc.tile_pool(name="psum", bufs=4, space="PSUM") as ps,
        ):
            iota_sb = cst.tile([P, Cmax * P], f16, tag="iota")
            nc.sync.dma_start(out=iota_sb[:], in_=iota.ap()[:, :])
            brep_sb = cst.tile([P, IN_CH], f32, tag="brep")
            nc.sync.dma_start(out=brep_sb[:], in_=brep.ap()[:, :])
            idx0_sb = cst.tile([P, max(1, L0 * 8)], i16, tag="idx0")
            nc.sync.dma_start(out=idx0_sb[:], in_=idx0.ap()[:, :])
            idx1_sb = cst.tile([P, max(1, L1 * 8)], i16, tag="idx1")
            nc.sync.dma_start(out=idx1_sb[:], in_=idx1.ap()[:, :])
            dstr_sb = cst.tile([P, CT], f16, tag="dstr")
            nc.sync.dma_start(out=dstr_sb[:], in_=dstr.ap()[:, :])
            alp_sb = cst.tile([P, CT * 4], f32, tag="alp")
            nc.sync.dma_start(out=alp_sb[:], in_=alp.ap()[:, :])

            # ev = exp(leaky_relu(alpha) + EXP_SHIFT) over the whole edge set
            t1 = cst.tile([P, CT * 4], f32, tag="t1")
            nc.vector.tensor_scalar(
                out=t1[:], in0=alp_sb[:], scalar1=NEG, scalar2=None,
                op0=mybir.AluOpType.mult,
            )
            nc.vector.tensor_tensor(
                out=t1[:], in0=alp_sb[:], in1=t1[:], op=mybir.AluOpType.max
            )
            shift_sb = cst.tile([P, 1], f32, tag="shift")
            nc.vector.memset(shift_sb[:], EXP_SHIFT)
            evt = cst.tile([P, CT * 4], f16, tag="evt")
            nc.scalar.activation(
                out=evt[:], in_=t1[:],
                func=mybir.ActivationFunctionType.Exp, bias=shift_sb[:],
            )

            # ensure gather-tile pads hold finite stale data, not SBUF garbage
            for _ in range(3):
                z0 = sb.tile([P, C0m, IN_CH], f16, tag="gtlo")
                nc.vector.memset(z0[:], 0.0)
                z1 = sb.tile([P, C1m, IN_CH], f16, tag="gthi")
                nc.vector.memset(z1[:], 0.0)

            cc = 0  # chunk column base
            o0 = o1 = 0  # idx slot bases (in chunks)
            for w in range(NW):
                c0, c1 = int(C0s[w]), int(C1s[w])
                CW = 1 + c0 + c1
                gtlo = sb.tile([P, C0m, IN_CH], f16, tag="gtlo")
                gthi = sb.tile([P, C1m, IN_CH], f16, tag="gthi")
                if c0 > 0:
                    nc.gpsimd.dma_gather(
                        out_ap=gtlo[:, 0:c0, :],
                        in_ap=G.ap()[0:HALF, :],
                        idxs_ap=idx0_sb[:, o0 * 8 : (o0 + c0) * 8],
                        num_idxs=c0 * P,
                        num_idxs_reg=c0 * P,
                        elem_size=IN_CH,
                        single_packet=(c0 * P <= 1024),
                    )
                if c1 > 0:
                    nc.gpsimd.dma_gather(
                        out_ap=gthi[:, 0:c1, :],
                        in_ap=G.ap()[HALF:NG, :],
                        idxs_ap=idx1_sb[:, o1 * 8 : (o1 + c1) * 8],
                        num_idxs=c1 * P,
                        num_idxs_reg=c1 * P,
                        elem_size=IN_CH,
                        single_packet=(c1 * P <= 1024),
                    )
                gs = sb.tile([P, 1, IN_CH], f16, tag="gs")
                nc.sync.dma_start(
                    out=gs[:, 0, :], in_=gselfd.ap()[P * w : P * (w + 1), :]
                )

                gts = sb.tile([P, Cmax, RH], f16, tag="gts")
                do_scale(gts, 0, 1, gs[:, 0:1, :], evt[:, cc * 4 : (cc + 1) * 4])
                if c0 > 0:
                    do_scale(
                        gts, 1, c0, gtlo[:, 0:c0, :],
                        evt[:, (cc + 1) * 4 : (cc + 1 + c0) * 4],
                    )
                if c1 > 0:
                    do_scale(
                        gts, 1 + c0, c1, gthi[:, 0:c1, :],
                        evt[:, (cc + 1 + c0) * 4 : (cc + CW) * 4],
                    )
                nc.vector.tensor_copy(
                    out=gts[:, 0:CW, IN_CH:RH],
                    in_=evt[:, cc * 4 : (cc + CW) * 4].rearrange(
                        "p (c h) -> p c h", h=HEADS
                    ),
                )

                oh = sb.tile([P, Cmax * P], f16, tag="oh")
                nc.vector.tensor_tensor(
                    out=oh[:, 0 : CW * P].rearrange("p (c n) -> p c n", n=P),
                    in0=iota_sb[:, 0 : CW * P].rearrange("p (c n) -> p c n", n=P),
                    in1=dstr_sb[:, cc : cc + CW]
                    .unsqueeze(2)
                    .to_broadcast([P, CW, P]),
                    op=mybir.AluOpType.is_equal,
                )

                pt = ps.tile([P, RH], f32, tag="pt")
                for c in range(CW):
                    nc.tensor.matmul(
                        out=pt[:],
                        lhsT=oh[:, c * P : (c + 1) * P],
                        rhs=gts[:, c, :],
                        start=(c == 0),
                        stop=(c == CW - 1),
                    )

                rcp = sb.tile([P, HEADS], f32, tag="rcp")
                nc.vector.reciprocal(out=rcp[:], in_=pt[:, IN_CH:RH])
                osb = sb.tile([P, IN_CH], f32, tag="osb")
                nc.vector.tensor_tensor(
                    out=osb[:].rearrange("p (h j) -> p h j", h=HEADS),
                    in0=pt[:, 0:IN_CH].rearrange("p (h j) -> p h j", h=HEADS),
                    in1=rcp[:].unsqueeze(2).to_broadcast([P, HEADS, OUT_CH]),
                    op=mybir.AluOpType.mult,
                )
                nc.vector.tensor_tensor(
                    out=osb[:], in0=osb[:], in1=brep_sb[:], op=mybir.AluOpType.add
                )
                of = sb.tile([P, IN_CH], f16, tag="of")
                nc.scalar.activation(
                    out=of[:], in_=osb[:], func=mybir.ActivationFunctionType.Relu
                )
                nc.sync.dma_start(
                    out=outd.ap()[P * w : P * (w + 1), :], in_=of[:]
                )
                cc += CW
                o0 += c0
                o1 += c1
    nc.compile()
    return nc


def _wrap16_concat(flat_vals, counts):
    """Per-window wrap-16 idx layout, concatenated along the free dim,
    replicated to 128 partitions. flat_vals: int16 slot values, window w
    occupying flat[128*cum[w] : 128*cum[w]+128*counts[w]]."""
    total_cols = max(1, int(sum(counts)) * 8)
    arr = np.zeros((16, total_cols), np.int16)
    off = 0
    col = 0
    for c in counts:
        c = int(c)
        blk = flat_vals[off : off + c * P].reshape(c * 8, 16).T
        arr[:, col : col + c * 8] = blk
        off += c * P
        col += c * 8
    return np.tile(arr, (8, 1))


def _prep_edges(edge_index, a_all):
    """Build per-core phase-2 input arrays from edge_index (no self loops;
    those are the implicit chunk 0 of each window) and the per-node attention
    projections a_all [NG, 8] (cols 0:4 a_src, 4:8 a_dst)."""
    src = edge_index[0].astype(np.int64)
    dst = edge_index[1].astype(np.int64)
    srcp = src + (NPCP - NPC) * (src // NPC)
    core = dst // NPC
    dloc = dst % NPC
    dwin = dloc // P
    drel = dloc % P
    dstp = NPCP * core + dloc
    half = (srcp >= HALF).astype(np.int64)  # 0 = lo, 1 = hi

    # per (core, window, half) counts -> shared static schedule
    cnt = np.zeros((M, NW, 2), np.int64)
    np.add.at(cnt, (core, dwin, half), 1)
    chunks = -(-cnt // P)  # ceil
    C0s = chunks[:, :, 0].max(axis=0)
    C1s = chunks[:, :, 1].max(axis=0)
    CWs = 1 + C0s + C1s
    base0 = np.concatenate([[0], np.cumsum(C0s)])  # in chunks
    base1 = np.concatenate([[0], np.cumsum(C1s)])
    basec = np.concatenate([[0], np.cumsum(CWs)])  # chunk cols incl self
    S0, S1 = int(base0[-1]) * P, int(base1[-1]) * P
    CT = int(basec[-1])

    # self-alpha per local node (vectorized over all cores)
    a_self = a_all[:, 0:4] + a_all[:, 4:8]  # [NG, 4]

    per_core = []
    for c in range(M):
        m = core == c
        ew, eh, er = dwin[m], half[m], drel[m]
        es, ed = srcp[m], dstp[m]
        order = np.lexsort((eh, ew))
        ew, eh, er, es, ed = (a[order] for a in (ew, eh, er, es, ed))
        gid = ew * 2 + eh
        uniq, start, cnts = np.unique(gid, return_index=True, return_counts=True)
        gstart = np.zeros(2 * NW, np.int64)
        gstart[uniq] = start
        rank = np.arange(len(gid)) - gstart[gid]
        # half-local slot (for idx arrays)
        shalf = np.where(eh == 0, base0[ew], base1[ew]) * P + rank
        # chunk-grid slot (for dstr / alpha): window-local chunk = 1 + ...
        ecol = basec[ew] + 1 + np.where(eh == 0, 0, C0s[ew]) + rank // P
        eslot = ecol * P + rank % P

        idx0f = np.zeros(S0, np.int16)
        idx1f = np.zeros(S1, np.int16)
        lo_m = eh == 0
        idx0f[shalf[lo_m]] = es[lo_m].astype(np.int16)
        idx1f[shalf[~lo_m]] = (es[~lo_m] - HALF).astype(np.int16)

        drf = np.full(CT * P, -1.0, np.float16)
        alf = np.zeros((CT * P, 4), np.float32)
        drf[eslot] = er.astype(np.float16)
        alf[eslot] = a_all[es, 0:4] + a_all[ed, 4:8]
        # self chunk 0 of each window: dstr = iota, alpha = a_self
        selfslots = (basec[:NW, None] * P + np.arange(P)[None, :]).ravel()
        drf[selfslots] = np.tile(np.arange(P, dtype=np.float16), NW)
        alf[selfslots] = a_self[NPCP * c : NPCP * (c + 1)]

        per_core.append(
            {
                "idx0": _wrap16_concat(idx0f, C0s),
                "idx1": _wrap16_concat(idx1f, C1s),
                "dstr": np.ascontiguousarray(drf.reshape(CT, P).T),
                "alp": np.ascontiguousarray(
                    alf.reshape(CT, P, 4).transpose(1, 0, 2).reshape(P, -1)
                ),
            }
        )
    return tuple(int(v) for v in C0s), tuple(int(v) for v in C1s), per_core


def kernel(x, edge_index, W, att_src, att_dst, bias):
    from concourse.bass_utils import run_bass_kernel_spmd

    x = np.asarray(x, dtype=np.float32)
    edge_index = np.asarray(edge_index).astype(np.int64)
    W = np.asarray(W, dtype=np.float32)
    att_src = np.asarray(att_src, dtype=np.float32)
    att_dst = np.asarray(att_dst, dtype=np.float32)
    bias = np.asarray(bias, dtype=np.float32)

    # host data layout prep
    x_pad = np.zeros((NG, IN_CH), np.float32)
    for c in range(M):
        x_pad[NPCP * c : NPCP * c + NPC] = x[NPC * c : NPC * c + NPC]
    xT_f16 = np.ascontiguousarray(x_pad.T).astype(np.float16)

    A8 = np.zeros((IN_CH, 2 * HEADS), np.float32)
    for h in range(HEADS):
        A8[OUT_CH * h : OUT_CH * (h + 1), h] = att_src[h]
        A8[OUT_CH * h : OUT_CH * (h + 1), HEADS + h] = att_dst[h]
    WC8 = np.concatenate([(W @ A8), W], axis=1).astype(np.float16)  # [256, 264]
    brep = np.tile(bias.astype(np.float32), (P, 1))

    # ---- program A: per-node h rows + attention projections ----
    if "A" in _cache:
        ncA = _cache["A"]
    else:
        ncA = _cache["A"] = _build_prog_a()
    in_maps_a = [
        {
            "xs": np.ascontiguousarray(xT_f16[:, NPCP * c : NPCP * (c + 1)]),
            "wc": WC8,
        }
        for c in range(M)
    ]
    res_a = run_bass_kernel_spmd(ncA, in_maps_a, core_ids=list(range(M)))
    a_all = np.concatenate([res_a.results[c]["ao"] for c in range(M)], axis=0)
    G = np.concatenate([res_a.results[c]["hs"] for c in range(M)], axis=0)
    G = np.ascontiguousarray(G)

    # ---- host: expand per-edge alpha, build schedule ----
    C0s, C1s, per_core = _prep_edges(edge_index, a_all)
    Cmax = max(1 + C0s[w] + C1s[w] for w in range(NW))
    iota = np.tile(np.arange(P, dtype=np.float16), (P, Cmax))

    key = ("B", C0s, C1s)
    if key in _cache:
        ncB = _cache[key]
    else:
        ncB = _cache[key] = _build_prog_b(C0s, C1s)
    in_maps_b = []
    for c in range(M):
        d = dict(per_core[c])
        d.update(
            {
                "G": G,
                "gself": np.ascontiguousarray(G[NPCP * c : NPCP * (c + 1)]),
                "iota": iota,
                "brep": brep,
            }
        )
        in_maps_b.append(d)
    res_b = run_bass_kernel_spmd(ncB, in_maps_b, core_ids=list(range(M)))

    out = np.empty((N, IN_CH), np.float32)
    for c in range(M):
        out[NPC * c : NPC * (c + 1)] = res_b.results[c]["out"][:NPC].astype(
            np.float32
        )
    _cache["last_run"] = (in_maps_a, in_maps_b)
    return out


def timed_run(np_inputs):
    """Re-run both programs with tracing; return summed max-core exec ns."""
    from concourse.bass_utils import run_bass_kernel_spmd

    if "last_run" not in _cache:
        kernel(**np_inputs)
    in_maps_a, in_maps_b = _cache["last_run"]
    ncA = _cache["A"]
    ncB = next(v for k, v in _cache.items() if isinstance(k, tuple) and k[0] == "B")
    total = 0
    for nc, im in ((ncA, in_maps_a), (ncB, in_maps_b)):
        r = run_bass_kernel_spmd(
            nc, im, core_ids=list(range(M)), trace=True
        )
        print("  exec_time_ns:", r.exec_time_ns, "trace:",
              r.instructions_and_trace[1] if r.instructions_and_trace else None)
        if r.exec_time_ns:
            total += r.exec_time_ns
    return total
